# revision 30
# baseline (speedup 1.0000x reference)
"""Multi-head attention (B=16, N=1024, D=768, H=12) on 8 TRN2 NeuronCores.

Strategy: pure data parallelism over the batch axis (2 batches per core, no
collectives). Per core, the whole attention block runs in bf16 matmuls with
f32 PSUM accumulation:

  - host pre-transposes x to x^T [B, D, N] and casts x / w_qkv / w_proj to
    bf16 (layout+dtype prep only; all FLOPs stay on device)
  - qkv^T = w_qkv^T @ x^T computed via PE (contract D on partitions), giving
    Q^T / K^T in [head_dim, n] layout directly; V is computed in natural
    [m, head_dim] layout (it is the PV matmul's stationary operand)
  - S^T[m, n] = K^T.T @ Q^T per head; the two heads of a pair run
    concurrently in the PE array via row tile_position (head_dim=64)
  - softmax without max-subtraction (scores are ~N(0,1); |S| < 9 measured),
    exp on ScalarE straight out of PSUM with the 1/sqrt(hd) scale folded in
  - PV uses lhsT = [V | ones] so each head's PSUM holds both the numerator
    O^T and 64 broadcast copies of the softmax denominator; normalization is
    a DVE approx-reciprocal + multiply, no partition reductions anywhere
  - out^T accumulates per head pair in [d, n] layout which feeds the final
    projection (contract D on partitions) producing [n, d] natural output

Scheduling: the attention inner loop is ACT(exp)-bound, so the next batch's
QKV/V matmul chunks and the previous batch's projection chunks are emitted
*inside* the attention loops (work queue), filling PE while ACT streams exps.
PSUM budget: psS (scores) 2x[128,1024]=4 banks, psO (out accum) 1x=2 banks,
psQ (interleaved qkv/proj chunks) 2x[128,512]=2 banks.
"""

import sys

if "/opt/trn_rl_repo" not in sys.path:
    sys.path.insert(0, "/opt/trn_rl_repo")

from contextlib import ExitStack

import ml_dtypes
import numpy as np

import concourse.bass as bass
import concourse.tile as tile
from concourse import bacc, mybir
from concourse.bass_utils import run_bass_kernel_spmd

N_CORES = 8
B, N, D = 16, 1024, 768
H, Hd = 12, 64
BPC = B // N_CORES  # batches per core
PAIRS = H // 2
NT = N // 128  # 8 token tiles of 128
DT = D // 128  # 6 contraction chunks of 128
SCALE = Hd**-0.5

BF16 = mybir.dt.bfloat16
F32 = mybir.dt.float32

_cached_nc = None


def _pbcast(ap, parts=128):
    """Broadcast a 1-D DRAM AP across `parts` partitions (partition step 0)."""
    return bass.AP(tensor=ap.tensor, offset=ap.offset, ap=[[0, parts]] + list(ap.ap))


def build_graph():
    nc = bacc.Bacc()
    xT = nc.declare_dram_parameter("xT", [BPC, D, N], BF16, isOutput=False)
    wq = nc.declare_dram_parameter("wq", [D, 3 * D], BF16, isOutput=False)
    wp = nc.declare_dram_parameter("wp", [D, D], BF16, isOutput=False)
    bq = nc.declare_dram_parameter("bq", [3 * D], F32, isOutput=False)
    bp = nc.declare_dram_parameter("bp", [D], F32, isOutput=False)
    out = nc.declare_dram_parameter("out", [BPC, N, D], F32, isOutput=True)

    with ExitStack() as ctx:
        tc = ctx.enter_context(tile.TileContext(nc))
        const = ctx.enter_context(tc.tile_pool(name="const", bufs=1))
        xt_pool = ctx.enter_context(tc.tile_pool(name="xt", bufs=2))
        qk_pool = ctx.enter_context(tc.tile_pool(name="qk", bufs=2))
        v_pool = ctx.enter_context(tc.tile_pool(name="v", bufs=2))
        ot_pool = ctx.enter_context(tc.tile_pool(name="ot", bufs=2))
        es_pool = ctx.enter_context(tc.tile_pool(name="es", bufs=4))
        sm_pool = ctx.enter_context(tc.tile_pool(name="sm", bufs=2))
        fo_pool = ctx.enter_context(tc.tile_pool(name="fo", bufs=2))
        psS = ctx.enter_context(tc.tile_pool(name="psS", bufs=2, space="PSUM"))
        psO = ctx.enter_context(tc.tile_pool(name="psO", bufs=1, space="PSUM"))
        psQ = ctx.enter_context(tc.tile_pool(name="psQ", bufs=2, space="PSUM"))

        # --- constants ---
        # weights load in thirds (Q, K, V column blocks) ordered by first use
        # so the first QK chunks' dependencies arrive as early as possible
        wq_sb = [const.tile([128, 3 * D], BF16, tag=f"wq{k}", name="wq") for k in range(DT)]
        for s in range(2):  # Q then K thirds
            for k in range(DT):
                nc.sync.dma_start(
                    out=wq_sb[k][:, D * s : D * (s + 1)],
                    in_=wq[128 * k : 128 * (k + 1), D * s : D * (s + 1)],
                )
        wp_sb = []
        for k in range(DT):
            t = const.tile([128, D], BF16, tag=f"wp{k}")
            wp_sb.append(t)
        # b_qkv rows of qkv^T are partitions: [128, 18] col r = b_qkv[128r:128(r+1)]
        bq_sb = const.tile([128, 18], F32, tag="bq")
        nc.gpsimd.dma_start(out=bq_sb, in_=bq[:].rearrange("(r p) -> p r", p=128))
        # free-axis biases broadcast across partitions (SWDGE handles stride-0)
        bpb = const.tile([128, D], F32, tag="bpb")
        nc.gpsimd.dma_start(out=bpb, in_=_pbcast(bp[:]))

        xt = [[None] * DT for _ in range(BPC)]
        qk_sb = [[None] * 12 for _ in range(BPC)]
        v_sb = [[None] * NT for _ in range(BPC)]
        ot_sb = [[None] * PAIRS for _ in range(BPC)]
        fo_ctr = [0]

        def emit_xt(b):
            for k in range(DT):
                t = xt_pool.tile([128, N], BF16, tag=f"xt{k}", name="xt")
                nc.sync.dma_start(out=t, in_=xT[b, 128 * k : 128 * (k + 1), :])
                xt[b][k] = t

        def emit_qk_chunk(b, r):
            # rows 128r:128(r+1) of qkv^T (Q^T for r<6, K^T for 6<=r<12)
            t = qk_pool.tile([128, N], BF16, tag=f"qk{r}", name="qk")
            for half in range(2):
                ps = psQ.tile([128, 512], F32, tag="psQ", name="psQ")
                for k in range(DT):
                    nc.tensor.matmul(
                        ps,
                        lhsT=wq_sb[k][:, 128 * r : 128 * (r + 1)],
                        rhs=xt[b][k][:, 512 * half : 512 * (half + 1)],
                        start=(k == 0),
                        stop=(k == DT - 1),
                    )
                nc.vector.tensor_scalar_add(
                    t[:, 512 * half : 512 * (half + 1)], ps, bq_sb[:, r : r + 1]
                )
            qk_sb[b][r] = t

        def emit_v_chunk(b, m):
            # V rows 128m:128(m+1) in natural [m, dv] layout, stored per pair
            # as [V_2p | ones | V_2p+1] (192 cols per pair)
            t = v_pool.tile([128, PAIRS * 192], BF16, tag=f"v{m}", name="v")
            tv = t.rearrange("p (a c) -> p a c", c=192)
            for n0, nw, p0, np_ in ((0, 512, 0, 4), (512, 256, 4, 2)):
                ps = psQ.tile([128, 512], F32, tag="psQ", name="psQ")
                for k in range(DT):
                    nc.tensor.matmul(
                        ps[:, 0:nw],
                        lhsT=xt[b][k][:, 128 * m : 128 * (m + 1)],
                        rhs=wq_sb[k][:, 2 * D + n0 : 2 * D + n0 + nw],
                        start=(k == 0),
                        stop=(k == DT - 1),
                    )
                # V bias is NOT added here: (O + b·sums)/sums = O/sums + b, so
                # it folds into a per-partition add after normalization
                pv = ps[:, 0:nw].rearrange("p (a c) -> p a c", c=128)
                nc.vector.tensor_copy(tv[:, p0 : p0 + np_, 0:64], pv[:, :, 0:64])
                nc.vector.tensor_copy(tv[:, p0 : p0 + np_, 128:192], pv[:, :, 64:128])
            nc.gpsimd.memset(tv[:, :, 64:128], 1.0)
            v_sb[b][m] = t

        def emit_proj_chunk(b, ti):
            # out[n, do] for token chunk ti: contract attn^T over di
            fo = fo_pool.tile([128, D], F32, tag="fo", name="fo")
            for n0, nw in ((0, 512), (512, 256)):
                ps = psQ.tile([128, 512], F32, tag="psQ", name="psQ")
                for p6 in range(DT):
                    nc.tensor.matmul(
                        ps[:, 0:nw],
                        lhsT=ot_sb[b][p6][:, 128 * ti : 128 * (ti + 1)],
                        rhs=wp_sb[p6][:, n0 : n0 + nw],
                        start=(p6 == 0),
                        stop=(p6 == DT - 1),
                    )
                nc.vector.tensor_add(
                    fo[:, n0 : n0 + nw], ps[:, 0:nw], bpb[:, n0 : n0 + nw]
                )
            nc.sync.dma_start(out=out[b, 128 * ti : 128 * (ti + 1), :], in_=fo)
            fo_ctr[0] += 1

        proj_part = [None] * NT

        def emit_proj_partial(b, ti, nparts):
            # first `nparts` di-chunks of proj accumulated early (+ bias),
            # parked as bf16 in the dead batch-0 qk slots (their last readers,
            # attention[0]'s matmuls, are long done by the time these run)
            part = qk_pool.tile([128, D], BF16, tag=f"qk{ti}", name="pp")
            for n0, nw in ((0, 512), (512, 256)):
                ps = psQ.tile([128, 512], F32, tag="psQ", name="psQ")
                for p6 in range(nparts):
                    nc.tensor.matmul(
                        ps[:, 0:nw],
                        lhsT=ot_sb[b][p6][:, 128 * ti : 128 * (ti + 1)],
                        rhs=wp_sb[p6][:, n0 : n0 + nw],
                        start=(p6 == 0),
                        stop=(p6 == nparts - 1),
                    )
                nc.vector.tensor_add(
                    part[:, n0 : n0 + nw], ps[:, 0:nw], bpb[:, n0 : n0 + nw]
                )
            proj_part[ti] = part

        def emit_proj_finish(b, ti, nparts):
            fo = fo_pool.tile([128, D], F32, tag="fo", name="fo")
            for n0, nw in ((0, 512), (512, 256)):
                ps = psQ.tile([128, 512], F32, tag="psQ", name="psQ")
                for p6 in range(nparts, DT):
                    nc.tensor.matmul(
                        ps[:, 0:nw],
                        lhsT=ot_sb[b][p6][:, 128 * ti : 128 * (ti + 1)],
                        rhs=wp_sb[p6][:, n0 : n0 + nw],
                        start=(p6 == nparts),
                        stop=(p6 == DT - 1),
                    )
                nc.vector.tensor_add(
                    fo[:, n0 : n0 + nw], ps[:, 0:nw], proj_part[ti][:, n0 : n0 + nw]
                )
            nc.sync.dma_start(out=out[b, 128 * ti : 128 * (ti + 1), :], in_=fo)

        rnd = [0]

        def emit_attention_pair(b, p, work, spacing=4):
            """Head pair (2p, 2p+1) of batch b. `work` is a list of closures
            (other-phase chunks) drained into the PE stream between rounds."""
            qt = qk_sb[b][p]
            kt = qk_sb[b][6 + p]
            ot = ot_pool.tile([128, N], BF16, tag=f"ot{p}", name="ot")
            for nhalf in range(2):
                pso = psO.tile([128, 1024], F32, tag="psO", name="psO")
                pv_prev = None
                for j in range(NT):
                    pss = psS.tile([128, 1024], F32, tag="psS", name="psS")
                    es = es_pool.tile([128, 1024], BF16, tag="es", name="es")
                    for h in range(2):
                        nc.tensor.matmul(
                            pss[:, 512 * h : 512 * (h + 1)],
                            lhsT=kt[64 * h : 64 * (h + 1), 128 * j : 128 * (j + 1)],
                            rhs=qt[
                                64 * h : 64 * (h + 1), 512 * nhalf : 512 * (nhalf + 1)
                            ],
                            start=True,
                            stop=True,
                            tile_position=(64 * h, 0),
                        )
                    # software pipeline: previous round's PV goes behind this
                    # round's S so PE never queues behind the exp wait
                    if pv_prev is not None:
                        pv_prev()
                    nc.scalar.activation(
                        out=es,
                        in_=pss,
                        func=mybir.ActivationFunctionType.Exp,
                        scale=SCALE,
                    )

                    def make_pv(j=j, es=es, pso=pso):
                        def emit():
                            for h in range(2):
                                nc.tensor.matmul(
                                    pso[:, 512 * h : 512 * (h + 1)],
                                    lhsT=v_sb[b][j][
                                        :, 192 * p + 64 * h : 192 * p + 64 * h + 128
                                    ],
                                    rhs=es[:, 512 * h : 512 * (h + 1)],
                                    start=(j == 0),
                                    stop=(j == NT - 1),
                                )

                        return emit

                    pv_prev = make_pv()
                    rnd[0] += 1
                    if work and rnd[0] % spacing == spacing - 1:
                        work.pop(0)()
                pv_prev()
                # cols 0:512 head A: O rows 0:64, sums rows 64:128
                # cols 512:1024 head B: sums rows 0:64, O rows 64:128.
                # One big copy releases psO after ~1.2us; the rest of the
                # normalization runs from SBUF off the PSUM critical path.
                # Custom-DVE ops only work at partition base 0, so recips are
                # base-0 and rcB is relocated with a plain cross-base copy;
                # the multiplies go to the otherwise-idle GpSimd (needs
                # matching SBUF base partitions, which this layout has).
                oc = sm_pool.tile([128, 1024], F32, tag="oc", name="oc")
                nc.vector.tensor_copy(oc, pso)
                smA = sm_pool.tile([64, 512], F32, tag="smA", name="smA")
                nc.vector.tensor_copy(smA, oc[64:128, 0:512])
                rcA = sm_pool.tile([64, 512], F32, tag="rcA", name="rcA")
                nc.vector.reciprocal_approx_fast(out=rcA, in_=smA)
                rcB = sm_pool.tile([128, 512], F32, tag="rcB", name="rcB")
                nc.vector.reciprocal_approx_fast(out=rcB[0:64, :], in_=oc[0:64, 512:1024])
                nc.vector.tensor_copy(rcB[64:128, :], rcB[0:64, :])
                nc.gpsimd.tensor_tensor(
                    ot[0:64, 512 * nhalf : 512 * (nhalf + 1)],
                    oc[0:64, 0:512],
                    rcA,
                    mybir.AluOpType.mult,
                )
                nc.gpsimd.tensor_tensor(
                    ot[64:128, 512 * nhalf : 512 * (nhalf + 1)],
                    oc[64:128, 512:1024],
                    rcB[64:128, :],
                    mybir.AluOpType.mult,
                )
                # deferred V bias (b_qkv's V section, per-partition in this
                # layout: bq_sb col 12+p rows = pair's head dims)
                nc.gpsimd.tensor_scalar_add(
                    ot[0:64, 512 * nhalf : 512 * (nhalf + 1)],
                    ot[0:64, 512 * nhalf : 512 * (nhalf + 1)],
                    bq_sb[0:64, 12 + p : 13 + p],
                )
                nc.gpsimd.tensor_scalar_add(
                    ot[64:128, 512 * nhalf : 512 * (nhalf + 1)],
                    ot[64:128, 512 * nhalf : 512 * (nhalf + 1)],
                    bq_sb[64:128, 12 + p : 13 + p],
                )
            ot_sb[b][p] = ot

        # ---- emission schedule ----
        # Solo head phase: only what attention[0] pairs 0-1 need up front.
        emit_xt(0)
        for r in (0, 6, 1, 7):
            emit_qk_chunk(0, r)
        # V third of w_qkv arrives behind the Q/K thirds + x^T
        for k in range(DT):
            nc.sync.dma_start(
                out=wq_sb[k][:, 2 * D : 3 * D],
                in_=wq[128 * k : 128 * (k + 1), 2 * D : 3 * D],
            )
        for m in range(NT):
            emit_v_chunk(0, m)
        emit_xt(1)
        # proj weights aren't needed until attention[1]'s interleaved work
        for k in range(DT):
            nc.sync.dma_start(out=wp_sb[k], in_=wp[128 * k : 128 * (k + 1), :])

        # attention[0] hides: batch-0's late-pair QK chunks (paced 2 pairs
        # ahead of use), then batch-1's early QK chunks and all of V[1].
        work = [lambda r=r: emit_qk_chunk(0, r) for pp in (2, 3, 4, 5) for r in (pp, 6 + pp)]
        work += [lambda r=r: emit_qk_chunk(1, r) for r in (0, 6, 1, 7)]
        work += [lambda m=m: emit_v_chunk(1, m) for m in range(NT)]
        for p in range(PAIRS):
            emit_attention_pair(0, p, work, spacing=4)
        while work:
            work.pop(0)()

        # attention[1] hides: batch-1's late-pair QK chunks (first — they are
        # needed soonest), proj[0], then proj[1] partials (di-chunks 0..3,
        # valid once attention[1] pairs 0..3 are emitted).
        work = [lambda r=r: emit_qk_chunk(1, r) for pp in (2, 3, 4, 5) for r in (pp, 6 + pp)]
        work += [lambda ti=ti: emit_proj_chunk(0, ti) for ti in range(NT)]
        work += [lambda ti=ti: emit_proj_partial(1, ti, 4) for ti in range(NT)]
        for p in range(PAIRS):
            emit_attention_pair(1, p, work, spacing=4)
        while work:
            work.pop(0)()

        for ti in range(NT):
            emit_proj_finish(1, ti, 4)

    nc.finalize()
    return nc


def _prep_inputs(x, w_qkv, b_qkv, w_proj, b_proj):
    xTv = np.ascontiguousarray(x.transpose(0, 2, 1)).astype(ml_dtypes.bfloat16)
    wqb = np.ascontiguousarray(w_qkv).astype(ml_dtypes.bfloat16)
    wpb = np.ascontiguousarray(w_proj).astype(ml_dtypes.bfloat16)
    bqf = np.ascontiguousarray(b_qkv).astype(np.float32)
    bpf = np.ascontiguousarray(b_proj).astype(np.float32)
    return [
        {
            "xT": xTv[BPC * i : BPC * (i + 1)],
            "wq": wqb,
            "wp": wpb,
            "bq": bqf,
            "bp": bpf,
        }
        for i in range(N_CORES)
    ]


def run(x, w_qkv, b_qkv, w_proj, b_proj, trace=False):
    global _cached_nc
    if _cached_nc is None:
        _cached_nc = build_graph()
    in_maps = _prep_inputs(x, w_qkv, b_qkv, w_proj, b_proj)
    res = run_bass_kernel_spmd(
        _cached_nc, in_maps, core_ids=list(range(N_CORES)), trace=trace
    )
    outp = np.concatenate(
        [np.asarray(res.results[i]["out"]) for i in range(N_CORES)], axis=0
    )
    return outp.astype(np.float32), res


def kernel(**inputs):
    outp, _ = run(
        inputs["x"],
        inputs["w_qkv"],
        inputs["b_qkv"],
        inputs["w_proj"],
        inputs["b_proj"],
    )
    return outp


# revision 34
# speedup vs baseline: 1.5769x; 1.5769x over previous
"""Multi-head attention (B=16, N=1024, D=768, H=12) on 8 TRN2 NeuronCores.

Strategy: pure data parallelism over the batch axis (2 batches per core, no
collectives). Per core, the whole attention block runs in bf16 matmuls with
f32 PSUM accumulation:

  - host pre-transposes x to x^T [B, D, N] and casts x / w_qkv / w_proj to
    bf16 (layout+dtype prep only; all FLOPs stay on device)
  - qkv^T = w_qkv^T @ x^T computed via PE (contract D on partitions), giving
    Q^T / K^T in [head_dim, n] layout directly; V is computed in natural
    [m, head_dim] layout (it is the PV matmul's stationary operand)
  - S^T[m, n] = K^T.T @ Q^T per head; the two heads of a pair run
    concurrently in the PE array via row tile_position (head_dim=64)
  - softmax without max-subtraction (scores are ~N(0,1); |S| < 9 measured),
    exp on ScalarE straight out of PSUM with the 1/sqrt(hd) scale folded in
  - PV uses lhsT = [V | ones] so each head's PSUM holds both the numerator
    O^T and 64 broadcast copies of the softmax denominator; normalization is
    a DVE approx-reciprocal + multiply, no partition reductions anywhere
  - out^T accumulates per head pair in [d, n] layout which feeds the final
    projection (contract D on partitions) producing [n, d] natural output

Scheduling: the attention inner loop is ACT(exp)-bound, so the next batch's
QKV/V matmul chunks and the previous batch's projection chunks are emitted
*inside* the attention loops (work queue), filling PE while ACT streams exps.
PSUM budget: psS (scores) 2x[128,1024]=4 banks, psO (out accum) 1x=2 banks,
psQ (interleaved qkv/proj chunks) 2x[128,512]=2 banks.
"""

import sys

if "/opt/trn_rl_repo" not in sys.path:
    sys.path.insert(0, "/opt/trn_rl_repo")

from contextlib import ExitStack

import ml_dtypes
import numpy as np

import concourse.bass as bass
import concourse.tile as tile
from concourse import bacc, mybir
from concourse.bass_utils import run_bass_kernel_spmd

N_CORES = 8
B, N, D = 16, 1024, 768
H, Hd = 12, 64
BPC = B // N_CORES  # batches per core
PAIRS = H // 2
NT = N // 128  # 8 token tiles of 128
DT = D // 128  # 6 contraction chunks of 128
SCALE = Hd**-0.5

BF16 = mybir.dt.bfloat16
F32 = mybir.dt.float32

_cached_nc = None


def _pbcast(ap, parts=128):
    """Broadcast a 1-D DRAM AP across `parts` partitions (partition step 0)."""
    return bass.AP(tensor=ap.tensor, offset=ap.offset, ap=[[0, parts]] + list(ap.ap))


def build_graph():
    nc = bacc.Bacc()
    xT = nc.declare_dram_parameter("xT", [BPC, D, N], BF16, isOutput=False)
    wq = nc.declare_dram_parameter("wq", [D, 3 * D], BF16, isOutput=False)
    wp = nc.declare_dram_parameter("wp", [D, D], BF16, isOutput=False)
    bq = nc.declare_dram_parameter("bq", [3 * D], F32, isOutput=False)
    bp = nc.declare_dram_parameter("bp", [D], F32, isOutput=False)
    out = nc.declare_dram_parameter("out", [BPC, N, D], F32, isOutput=True)

    with ExitStack() as ctx:
        tc = ctx.enter_context(tile.TileContext(nc))
        const = ctx.enter_context(tc.tile_pool(name="const", bufs=1))
        xt_pool = ctx.enter_context(tc.tile_pool(name="xt", bufs=2))
        qk_pool = ctx.enter_context(tc.tile_pool(name="qk", bufs=2))
        v_pool = ctx.enter_context(tc.tile_pool(name="v", bufs=2))
        ot_pool = ctx.enter_context(tc.tile_pool(name="ot", bufs=2))
        es_pool = ctx.enter_context(tc.tile_pool(name="es", bufs=3))
        sm_pool = ctx.enter_context(tc.tile_pool(name="sm", bufs=2))
        fo_pool = ctx.enter_context(tc.tile_pool(name="fo", bufs=2))
        psS = ctx.enter_context(tc.tile_pool(name="psS", bufs=2, space="PSUM"))
        psO = ctx.enter_context(tc.tile_pool(name="psO", bufs=1, space="PSUM"))
        psQ = ctx.enter_context(tc.tile_pool(name="psQ", bufs=2, space="PSUM"))

        # --- constants ---
        # weights load in thirds (Q, K, V column blocks) ordered by first use
        # so the first QK chunks' dependencies arrive as early as possible
        wq_sb = [const.tile([128, 3 * D], BF16, tag=f"wq{k}", name="wq") for k in range(DT)]
        for s in range(2):  # Q then K thirds
            for k in range(DT):
                nc.sync.dma_start(
                    out=wq_sb[k][:, D * s : D * (s + 1)],
                    in_=wq[128 * k : 128 * (k + 1), D * s : D * (s + 1)],
                )
        wp_sb = []
        for k in range(DT):
            t = const.tile([128, D], BF16, tag=f"wp{k}")
            wp_sb.append(t)
        # b_qkv rows of qkv^T are partitions: [128, 18] col r = b_qkv[128r:128(r+1)]
        bq_sb = const.tile([128, 18], F32, tag="bq")
        nc.gpsimd.dma_start(out=bq_sb, in_=bq[:].rearrange("(r p) -> p r", p=128))
        # free-axis biases broadcast across partitions (SWDGE handles stride-0)
        bpb = const.tile([128, D], F32, tag="bpb")
        nc.gpsimd.dma_start(out=bpb, in_=_pbcast(bp[:]))
        bvb = const.tile([128, D], F32, tag="bvb")
        nc.gpsimd.dma_start(out=bvb, in_=_pbcast(bq[2 * D : 3 * D]))

        xt = [[None] * DT for _ in range(BPC)]
        qk_sb = [[None] * 12 for _ in range(BPC)]
        v_sb = [[None] * NT for _ in range(BPC)]
        ot_sb = [[None] * PAIRS for _ in range(BPC)]
        fo_ctr = [0]

        def emit_xt(b):
            for k in range(DT):
                t = xt_pool.tile([128, N], BF16, tag=f"xt{k}", name="xt")
                nc.sync.dma_start(out=t, in_=xT[b, 128 * k : 128 * (k + 1), :])
                xt[b][k] = t

        def emit_qk_chunk(b, r):
            # rows 128r:128(r+1) of qkv^T (Q^T for r<6, K^T for 6<=r<12)
            t = qk_pool.tile([128, N], BF16, tag=f"qk{r}", name="qk")
            for half in range(2):
                ps = psQ.tile([128, 512], F32, tag="psQ", name="psQ")
                for k in range(DT):
                    nc.tensor.matmul(
                        ps,
                        lhsT=wq_sb[k][:, 128 * r : 128 * (r + 1)],
                        rhs=xt[b][k][:, 512 * half : 512 * (half + 1)],
                        start=(k == 0),
                        stop=(k == DT - 1),
                    )
                nc.vector.tensor_scalar_add(
                    t[:, 512 * half : 512 * (half + 1)], ps, bq_sb[:, r : r + 1]
                )
            qk_sb[b][r] = t

        def emit_v_chunk(b, m):
            # V rows 128m:128(m+1) in natural [m, dv] layout, stored per pair
            # as [V_2p | ones | V_2p+1] (192 cols per pair)
            t = v_pool.tile([128, PAIRS * 192], BF16, tag=f"v{m}", name="v")
            tv = t.rearrange("p (a c) -> p a c", c=192)
            for n0, nw, p0, np_ in ((0, 512, 0, 4), (512, 256, 4, 2)):
                ps = psQ.tile([128, 512], F32, tag="psQ", name="psQ")
                for k in range(DT):
                    nc.tensor.matmul(
                        ps[:, 0:nw],
                        lhsT=xt[b][k][:, 128 * m : 128 * (m + 1)],
                        rhs=wq_sb[k][:, 2 * D + n0 : 2 * D + n0 + nw],
                        start=(k == 0),
                        stop=(k == DT - 1),
                    )
                pv = ps[:, 0:nw].rearrange("p (a c) -> p a c", c=128)
                bv = bvb[:, n0 : n0 + nw].rearrange("p (a c) -> p a c", c=128)
                nc.vector.tensor_add(
                    tv[:, p0 : p0 + np_, 0:64], pv[:, :, 0:64], bv[:, :, 0:64]
                )
                nc.vector.tensor_add(
                    tv[:, p0 : p0 + np_, 128:192], pv[:, :, 64:128], bv[:, :, 64:128]
                )
            nc.gpsimd.memset(tv[:, :, 64:128], 1.0)
            v_sb[b][m] = t

        def emit_proj_chunk(b, ti):
            # out[n, do] for token chunk ti: contract attn^T over di
            fo = fo_pool.tile([128, D], F32, tag="fo", name="fo")
            for n0, nw in ((0, 512), (512, 256)):
                ps = psQ.tile([128, 512], F32, tag="psQ", name="psQ")
                for p6 in range(DT):
                    nc.tensor.matmul(
                        ps[:, 0:nw],
                        lhsT=ot_sb[b][p6][:, 128 * ti : 128 * (ti + 1)],
                        rhs=wp_sb[p6][:, n0 : n0 + nw],
                        start=(p6 == 0),
                        stop=(p6 == DT - 1),
                    )
                nc.vector.tensor_add(
                    fo[:, n0 : n0 + nw], ps[:, 0:nw], bpb[:, n0 : n0 + nw]
                )
            nc.sync.dma_start(out=out[b, 128 * ti : 128 * (ti + 1), :], in_=fo)
            fo_ctr[0] += 1

        proj_part = [None] * NT

        def emit_proj_partial(b, ti, nparts):
            # first `nparts` di-chunks of proj accumulated early (+ bias),
            # parked as bf16 in the dead batch-0 qk slots (their last readers,
            # attention[0]'s matmuls, are long done by the time these run)
            part = qk_pool.tile([128, D], BF16, tag=f"qk{ti}", name="pp")
            for n0, nw in ((0, 512), (512, 256)):
                ps = psQ.tile([128, 512], F32, tag="psQ", name="psQ")
                for p6 in range(nparts):
                    nc.tensor.matmul(
                        ps[:, 0:nw],
                        lhsT=ot_sb[b][p6][:, 128 * ti : 128 * (ti + 1)],
                        rhs=wp_sb[p6][:, n0 : n0 + nw],
                        start=(p6 == 0),
                        stop=(p6 == nparts - 1),
                    )
                nc.vector.tensor_add(
                    part[:, n0 : n0 + nw], ps[:, 0:nw], bpb[:, n0 : n0 + nw]
                )
            proj_part[ti] = part

        def emit_proj_finish(b, ti, nparts):
            fo = fo_pool.tile([128, D], F32, tag="fo", name="fo")
            for n0, nw in ((0, 512), (512, 256)):
                ps = psQ.tile([128, 512], F32, tag="psQ", name="psQ")
                for p6 in range(nparts, DT):
                    nc.tensor.matmul(
                        ps[:, 0:nw],
                        lhsT=ot_sb[b][p6][:, 128 * ti : 128 * (ti + 1)],
                        rhs=wp_sb[p6][:, n0 : n0 + nw],
                        start=(p6 == nparts),
                        stop=(p6 == DT - 1),
                    )
                nc.vector.tensor_add(
                    fo[:, n0 : n0 + nw], ps[:, 0:nw], proj_part[ti][:, n0 : n0 + nw]
                )
            nc.sync.dma_start(out=out[b, 128 * ti : 128 * (ti + 1), :], in_=fo)

        rnd = [0]

        def emit_attention_pair(b, p, work, spacing=4):
            """Head pair (2p, 2p+1) of batch b. `work` is a list of closures
            (other-phase chunks) drained into the PE stream between rounds."""
            qt = qk_sb[b][p]
            kt = qk_sb[b][6 + p]
            ot = ot_pool.tile([128, N], BF16, tag=f"ot{p}", name="ot")
            for nhalf in range(2):
                pso = psO.tile([128, 1024], F32, tag="psO", name="psO")
                pv_prev = None
                for j in range(NT):
                    pss = psS.tile([128, 1024], F32, tag="psS", name="psS")
                    es = es_pool.tile([128, 1024], BF16, tag="es", name="es")
                    for h in range(2):
                        nc.tensor.matmul(
                            pss[:, 512 * h : 512 * (h + 1)],
                            lhsT=kt[64 * h : 64 * (h + 1), 128 * j : 128 * (j + 1)],
                            rhs=qt[
                                64 * h : 64 * (h + 1), 512 * nhalf : 512 * (nhalf + 1)
                            ],
                            start=True,
                            stop=True,
                            tile_position=(64 * h, 0),
                        )
                    # software pipeline: previous round's PV goes behind this
                    # round's S so PE never queues behind the exp wait
                    if pv_prev is not None:
                        pv_prev()
                    nc.scalar.activation(
                        out=es,
                        in_=pss,
                        func=mybir.ActivationFunctionType.Exp,
                        scale=SCALE,
                    )

                    def make_pv(j=j, es=es, pso=pso):
                        def emit():
                            for h in range(2):
                                nc.tensor.matmul(
                                    pso[:, 512 * h : 512 * (h + 1)],
                                    lhsT=v_sb[b][j][
                                        :, 192 * p + 64 * h : 192 * p + 64 * h + 128
                                    ],
                                    rhs=es[:, 512 * h : 512 * (h + 1)],
                                    start=(j == 0),
                                    stop=(j == NT - 1),
                                )

                        return emit

                    pv_prev = make_pv()
                    rnd[0] += 1
                    if work and rnd[0] % spacing == spacing - 1:
                        work.pop(0)()
                pv_prev()
                # cols 0:512 head A: O rows 0:64, sums rows 64:128
                # cols 512:1024 head B: sums rows 0:64, O rows 64:128.
                # One big copy releases psO after ~1.2us; the rest of the
                # normalization runs from SBUF off the PSUM critical path.
                # Custom-DVE ops only work at partition base 0, so recips are
                # base-0 and rcB is relocated with a plain cross-base copy;
                # the multiplies go to the otherwise-idle GpSimd (needs
                # matching SBUF base partitions, which this layout has).
                oc = sm_pool.tile([128, 1024], F32, tag="oc", name="oc")
                nc.vector.tensor_copy(oc, pso)
                smA = sm_pool.tile([64, 512], F32, tag="smA", name="smA")
                nc.vector.tensor_copy(smA, oc[64:128, 0:512])
                rcA = sm_pool.tile([64, 512], F32, tag="rcA", name="rcA")
                nc.vector.reciprocal_approx_fast(out=rcA, in_=smA)
                rcB = sm_pool.tile([128, 512], F32, tag="rcB", name="rcB")
                nc.vector.reciprocal_approx_fast(out=rcB[0:64, :], in_=oc[0:64, 512:1024])
                nc.vector.tensor_copy(rcB[64:128, :], rcB[0:64, :])
                nc.gpsimd.tensor_tensor(
                    ot[0:64, 512 * nhalf : 512 * (nhalf + 1)],
                    oc[0:64, 0:512],
                    rcA,
                    mybir.AluOpType.mult,
                )
                nc.gpsimd.tensor_tensor(
                    ot[64:128, 512 * nhalf : 512 * (nhalf + 1)],
                    oc[64:128, 512:1024],
                    rcB[64:128, :],
                    mybir.AluOpType.mult,
                )
            ot_sb[b][p] = ot

        # ---- emission schedule ----
        # Solo head phase: only what attention[0] pairs 0-1 need up front.
        emit_xt(0)
        for r in (0, 6, 1, 7):
            emit_qk_chunk(0, r)
        # V third of w_qkv arrives behind the Q/K thirds + x^T
        for k in range(DT):
            nc.sync.dma_start(
                out=wq_sb[k][:, 2 * D : 3 * D],
                in_=wq[128 * k : 128 * (k + 1), 2 * D : 3 * D],
            )
        for m in range(NT):
            emit_v_chunk(0, m)
        emit_xt(1)
        # proj weights aren't needed until attention[1]'s interleaved work
        for k in range(DT):
            nc.sync.dma_start(out=wp_sb[k], in_=wp[128 * k : 128 * (k + 1), :])

        # attention[0] hides: batch-0's late-pair QK chunks (paced 2 pairs
        # ahead of use), then batch-1's early QK chunks and all of V[1].
        work = [lambda r=r: emit_qk_chunk(0, r) for pp in (2, 3, 4, 5) for r in (pp, 6 + pp)]
        work += [lambda r=r: emit_qk_chunk(1, r) for r in (0, 6, 1, 7)]
        work += [lambda m=m: emit_v_chunk(1, m) for m in range(NT)]
        for p in range(PAIRS):
            emit_attention_pair(0, p, work, spacing=4)
        while work:
            work.pop(0)()

        # attention[1] hides: batch-1's late-pair QK chunks (first — they are
        # needed soonest), proj[0], then proj[1] partials (di-chunks 0..3,
        # valid once attention[1] pairs 0..3 are emitted).
        work = [lambda r=r: emit_qk_chunk(1, r) for pp in (2, 3, 4, 5) for r in (pp, 6 + pp)]
        work += [lambda ti=ti: emit_proj_chunk(0, ti) for ti in range(NT)]
        work += [lambda ti=ti: emit_proj_partial(1, ti, 4) for ti in range(NT)]
        for p in range(PAIRS):
            emit_attention_pair(1, p, work, spacing=4)
        while work:
            work.pop(0)()

        for ti in range(NT):
            emit_proj_finish(1, ti, 4)

    nc.finalize()
    return nc


def _prep_inputs(x, w_qkv, b_qkv, w_proj, b_proj):
    xTv = np.ascontiguousarray(x.transpose(0, 2, 1)).astype(ml_dtypes.bfloat16)
    wqb = np.ascontiguousarray(w_qkv).astype(ml_dtypes.bfloat16)
    wpb = np.ascontiguousarray(w_proj).astype(ml_dtypes.bfloat16)
    bqf = np.ascontiguousarray(b_qkv).astype(np.float32)
    bpf = np.ascontiguousarray(b_proj).astype(np.float32)
    return [
        {
            "xT": xTv[BPC * i : BPC * (i + 1)],
            "wq": wqb,
            "wp": wpb,
            "bq": bqf,
            "bp": bpf,
        }
        for i in range(N_CORES)
    ]


def run(x, w_qkv, b_qkv, w_proj, b_proj, trace=False):
    global _cached_nc
    if _cached_nc is None:
        _cached_nc = build_graph()
    in_maps = _prep_inputs(x, w_qkv, b_qkv, w_proj, b_proj)
    res = run_bass_kernel_spmd(
        _cached_nc, in_maps, core_ids=list(range(N_CORES)), trace=trace
    )
    outp = np.concatenate(
        [np.asarray(res.results[i]["out"]) for i in range(N_CORES)], axis=0
    )
    return outp.astype(np.float32), res


def kernel(**inputs):
    outp, _ = run(
        inputs["x"],
        inputs["w_qkv"],
        inputs["b_qkv"],
        inputs["w_proj"],
        inputs["b_proj"],
    )
    return outp
